# revision 54
# baseline (speedup 1.0000x reference)
"""LocalConv2D (3x3, width split into 4 weight blocks, 4-bit fake-quant weights)
on 8 Trainium2 NeuronCores.

Strategy
--------
Data-parallel over batch: 32 images -> 4 per core, processed as 2 pairs.
Image A of a pair lives in SBUF partitions 0-63 (its 64 channels), image B
in partitions 64-127. The 3x3 conv is 9 shifted K=64 matmuls accumulated in
PSUM; A's matmuls run in PE row-group 0 and B's in row-group 64
(tile_position auto-derived). The two K=64 matmuls of a tap stream
CONCURRENTLY through the PE array (disjoint row groups), so a tap pair
costs ~N cycles total.

Trace-derived facts this version is built around (17 profiled iterations):
- The HAM clock un-throttles (1.2->2.4 GHz) only after one fully-busy
  4096-cycle window of FULL-ARRAY activity. K=64 single-row-group warmup
  does NOT count as busy; warmup here runs as row-group PAIRS from the
  earliest possible instant (~7.5us) and is sized to bridge seamlessly to
  the first real matmul (~12.5us) — a PE idle >=1.5us there re-throttles
  the clock for ~3.4us (measured repeatedly).
- An A/B pair's PSUM banks must not coincide mod 4 or the concurrent
  drains serialize (~+35ns/pair); tag layout keeps them 3 apart.
- DMA queue first-data latency: SP ~1.5us stable; ACT 1.5-3.0us run-
  variable. A DMA's completion semaphore lags its data by ~0.4us at queue
  depth 1 growing to 2-5us at depth 3+ (high variance). Hence: coarse
  whole-tensor transfers; everything PE-gating at depth <=2 (q0 strips
  lead SP, b0 weights lead ACT as ONE chunk); depth >=3 slots hold only
  items with >=5us slack (b1 weights, bias, q1 strips, b2/b3 weights).
- Group order interleaves hh within each b so each arriving strip/weight
  chunk is consumed at half rate, doubling every DMA-arrival margin. The
  first group is h-split into two 14-row tiles for the earliest start.
- Warm steady state measures ~164ns per tap pair = the N/2.4GHz stream
  floor. The graded window also includes a fixed ~8.5us walrus epilogue
  (semaphore resets) that no kernel structure can avoid.

Weights are fake-quantized per-tensor to 4 bits: q = round(w/s)*s with
s = max|w|/7. round(w/s) is a small integer in [-7,7], exactly
representable in bf16, so the matmul runs on exact integer weights and the
scale s is folded into the eviction (out = psum*s + bias). Output staged
bf16 (fro-error ~2.3e-3, well under the 2e-2 gate).
"""

import numpy as np

KSIZE = 3
SW = 4
KBITS = 4
N, C, H, W, F = 32, 64, 56, 56, 128
HP, WP = H + 2, W + 2          # padded 58x58
N_CORES = 8
IMGS_PER_CORE = N // N_CORES   # 4
PAIRS = IMGS_PER_CORE // 2     # 2
WB = W // SW                   # 14
HH = H // 2                    # 28 out rows per h-half tile
HR = HH + 2                    # 30 input rows feeding one h-half
SUB = HH // 2                  # 14 out rows per h-quarter sub-tile
SUBR = SUB + 2                 # 16 input rows feeding a sub-tile

NT = HH * WB                   # 392 free elems per full tile
NS = SUB * WB                  # 196 free elems per sub tile

W_N = 392                      # warmup moving size (cold pair ~327ns)
W_SHORT = 3                    # short (N=128) warmup pairs while memset part 2 lands
W_LONG = 16                    # long (N=392) warmup pairs

_COMPILED = {}


def _install_drain_patch():
    """The walrus build here rejects instructions carrying >2 sync waits
    ('Too many sync wait commands'). Two fixes, both relying on engines
    executing their own stream in order:

    1. _add_instruction: any scheduled instruction with >2 waits gets
       same-engine NoOps inserted before it, each carrying <=2 of the waits.
    2. The Tile tail drain gets one wait per outstanding logical proc; emit
       one SP nop per proc, then strip the duplicated waits off the drain.
    """
    import re
    import bass_rust
    from concourse.vector_clock import ScopedClock
    import concourse.tile as tile
    import concourse.mybir as mybir

    if getattr(tile.TileContext, "_drain_patch_installed", False):
        return

    MAXW = 1       # this walrus build fits exactly 1 sync wait per instruction
    NOP_MAXW = 1
    _orig_add = tile.TileContext._add_instruction

    def _add_split(self, inst):
        si = getattr(inst, "sync_info", None)
        if si is not None and si.on_wait and len(si.on_wait) > MAXW:
            waits = list(si.on_wait)
            while len(waits) > MAXW:
                chunk, waits = waits[:NOP_MAXW], waits[NOP_MAXW:]
                nop = mybir.InstNoOp(
                    name=self.nc.get_next_instruction_name(), ins=[], outs=[]
                )
                nop.engine = inst.engine
                nop.sync_info = bass_rust.SyncInfo(on_wait=chunk, on_update=[])
                _orig_add(self, nop)
            si.on_wait = waits
        return _orig_add(self, inst)

    tile.TileContext._add_instruction = _add_split

    _orig = tile.TileContext._drain_and_barrier

    def _split(self, tick_clock, wait_clock):
        gc = tick_clock.global_clock
        m = re.match(r"VectorClock\(\[(.*)\]\)", repr(gc))
        vals = [int(v) for v in m.group(1).split(",")] if m.group(1).strip() else []
        covered = set()
        # Round-robin the per-proc wait nops across engines so they wait in
        # parallel (serial on one engine costs ~55ns each x ~30 procs). The
        # all-engine barrier that _orig emits right after collects them.
        engines = [
            self.nc.sync, self.nc.scalar, self.nc.vector,
            self.nc.gpsimd, self.nc.tensor,
        ]
        eng_i = 0
        for i, v in enumerate(vals):
            if v == 0:
                continue
            part = [0] * len(vals)
            part[i] = v
            nop = engines[eng_i % len(engines)].nop()
            eng_i += 1
            wait_clock.add_sem_waits(
                nop.ins, ScopedClock({None: bass_rust.VectorClock(part)})
            )
            si = nop.ins.sync_info
            for w in (si.on_wait if si else []) or []:
                covered.add((w.ant_name, w.wait_value))
        holder = []
        orig_drain = self.nc.sync.drain

        def capture(*a, **k):
            inst = orig_drain(*a, **k)
            holder.append(inst)
            return inst

        self.nc.sync.drain = capture
        try:
            r = _orig(self, tick_clock, wait_clock)
        finally:
            self.nc.sync.drain = orig_drain
        if holder:
            inst = holder[0].ins if hasattr(holder[0], "ins") else holder[0]
            si = inst.sync_info
            if si and si.on_wait:
                si.on_wait = [
                    w for w in si.on_wait
                    if (w.ant_name, w.wait_value) not in covered
                ]
        return r

    tile.TileContext._drain_and_barrier = _split
    tile.TileContext._drain_patch_installed = True


def _build_program():
    import concourse.bass as bass
    import concourse.mybir as mybir
    import concourse.tile as tile

    _install_drain_patch()

    f32 = mybir.dt.float32
    bf16 = mybir.dt.bfloat16

    nc = bass.Bass(target_bir_lowering=False, debug=False)
    # per-core inputs: image pairs packed [128, HP*WP] (A in 0-63, B 64-127)
    xin = nc.declare_dram_parameter(
        "xin", [PAIRS, 128, HP * WP], bf16, isOutput=False
    )
    # weights duplicated across halves: col (b*9 + t)*F + f
    wts = nc.declare_dram_parameter("wts", [128, SW * 9 * F], bf16, isOutput=False)
    bsc_p = nc.declare_dram_parameter("bsc", [128, 2], f32, isOutput=False)
    # b-major output layout: block (b, hh) of an image is contiguous, so
    # each PSUM eviction can stream straight to DRAM. Host reassembles.
    y = nc.declare_dram_parameter(
        "y", [IMGS_PER_CORE, 128, SW, H * WB], bf16, isOutput=True
    )

    WCH = 9 * F                  # weight columns per b-block

    with tile.TileContext(nc) as tc:
        with (
            tc.tile_pool(name="consts", bufs=1) as consts,
            tc.tile_pool(name="strips", bufs=1) as strips,
            tc.tile_pool(name="stage", bufs=8) as stage,
            tc.tile_pool(name="psum", bufs=4, space="PSUM") as psum,
        ):
            # Dummy source for warmup matmuls. Memset on GpSimd (otherwise
            # idle) in two stages: a small first chunk so the first LDW can
            # issue ~0.3us after the preamble barrier, the rest behind it.
            warm = consts.tile([128, W_N], bf16)
            nc.gpsimd.memset(warm[:, 0:F], 0.0)
            nc.gpsimd.memset(warm[:, F:W_N], 0.0)

            wt = consts.tile([128, SW * 9 * F], bf16)
            bsc_t = consts.tile([128, 2], f32)

            # --- input DMA schedule (order = queue order) ---
            # All of b0's weights must be resident before the first real
            # matmul: a mid-group weight wait stalls the PE long enough to
            # re-throttle the HAM clock (measured -6us). Split b0 across
            # both queues so the slower ACT ring doesn't gate, and lead SP
            # with taps 0-4 ahead of the strips.
            s = {}
            for q in range(PAIRS):
                for hh in range(2):
                    st = strips.tile([128, HR * WP], bf16, tag=f"s{q}{hh}")
                    s[(q, hh)] = st
            # Coarse whole-tensor transfers: the queues' data rate degrades
            # and completion semaphores lag by 1-3us when many DMAs are
            # queued, so few big DMAs with wide consumption margins beat
            # fine-grained racing (measured across three schedule attempts).
            # SP ring (fast: ~1.4us first-data, lower completion-sem lag):
            # the two deadline-critical q0 strips, then b2/b3 weights.
            # ACT ring (slow: ~3us first-data, sems lag up to ~2us): tiny
            # bias/scale first, b0 weights in two chunks (first chunk's sem
            # unblocks the first matmul ~1us sooner than one 295KB DMA),
            # b1 weights, then the q1 strips whose deadlines are ~12us out.
            # Outputs ride the same two rings; the deep `stage` pool
            # absorbs the out-DMA backlog while inputs still stream.
            # Queue facts (measured over ten runs): first-data latency is
            # ~1.5us on SP but 1.5-3.0us run-variable on ACT; a DMA's
            # completion semaphore lags its data by ~0.4us at queue depth 1
            # growing to ~2us at depth 3+. So: the first matmul's needs are
            # split across BOTH queue heads (b0 taps 0-4 lead SP, taps 5-8
            # lead ACT), strips follow on SP, and everything at depth >=3
            # on either queue has a deadline >=4us out.
            # b1's weights at SP depth 2 (sem ~12.4-13.5, well before their
            # ~14-15us deadline); s01 at depth 3 with its consumer pushed
            # one group later so both margins stay positive. bias/scale
            # behind them — its consumer (first eviction) has a soft
            # deadline: three PSUM banks and eight stage slots of cushion
            # absorb a late arrival without stalling the PE.
            nc.sync.dma_start(s[(0, 0)][:], xin[0, :, 0:HR * WP])
            nc.sync.dma_start(wt[:, WCH:2 * WCH], wts[:, WCH:2 * WCH])
            nc.sync.dma_start(
                s[(0, 1)][:], xin[0, :, HH * WP:(HH + HR) * WP]
            )
            nc.sync.dma_start(bsc_t[:], bsc_p[:])
            nc.sync.dma_start(s[(1, 0)][:], xin[1, :, 0:HR * WP])
            nc.sync.dma_start(
                s[(1, 1)][:], xin[1, :, HH * WP:(HH + HR) * WP]
            )
            # ACT carries only weights with >=6us slack, in consumption
            # order. b0 as ONE chunk: a split chunk puts taps 5-8 at queue
            # depth 2, whose completion sem can lag to ~17us and stall
            # mid-group.
            nc.scalar.dma_start(wt[:, 0:WCH], wts[:, 0:WCH])
            nc.scalar.dma_start(wt[:, 2 * WCH:3 * WCH], wts[:, 2 * WCH:3 * WCH])
            nc.scalar.dma_start(wt[:, 3 * WCH:4 * WCH], wts[:, 3 * WCH:4 * WCH])
            # Shifted strip copies for aligned j=1 taps: an odd-element
            # moving base costs +23ns per matmul pair (a third of all
            # taps). The shifted copy is the SAME DRAM at offset +1 element
            # DMA'd into an even-aligned SBUF tile — no host prep, no
            # engine copies. They ride the otherwise-idle ACT tail and are
            # consumed only by groups whose start is >=3us past the
            # worst-case arrival (late q0-hh1 groups and all of q1).
            sx = {}
            for (q, hh) in [(0, 1), (1, 0), (1, 1)]:
                st = strips.tile(
                    [128, HR * WP], bf16, tag=f"x{q}{hh}", name=f"sx{q}{hh}"
                )
                sx[(q, hh)] = st
                off = hh * HH * WP
                nc.scalar.dma_start(
                    st[:, 0:HR * WP - 1],
                    xin[q, :, off + 1:off + HR * WP],
                )

            # --- PE warmup: full-array paired matmuls, no DMA dependency ---
            # PSUM slot layout note: tag "W" occupies banks 0-1, "A" banks
            # 2-4, "B" banks 5-7. An A/B pair is always 3 banks apart —
            # pairs whose banks coincide mod 4 measurably serialize their
            # concurrent drains (~+35ns/pair warm), so keep the distance 3.
            # One long accumulation chain per bank: no inter-group WAW
            # stalls. First pairs are short (N=128) so they only need the
            # first memset chunk.
            wa = psum.tile([128, W_N], f32, tag="W", name="wa", bufs=2)
            wb = psum.tile([128, W_N], f32, tag="W", name="wb", bufs=2)
            for k in range(W_SHORT + W_LONG):
                first, last = k == 0, k == W_SHORT + W_LONG - 1
                n = F if k < W_SHORT else W_N
                nc.tensor.matmul(
                    wa[:, 0:n], warm[0:64, 0:F], warm[0:64, 0:n],
                    start=first, stop=last,
                )
                nc.tensor.matmul(
                    wb[:, 0:n], warm[64:128, 0:F], warm[64:128, 0:n],
                    start=first, stop=last,
                )

            def emit_tile(q, hh, b, r0, nr, shifted=False):
                """One output tile: image pair q, h-half hh, width block b,
                out rows [r0, r0+nr) of the half. Returns nothing; emits
                matmuls + eviction + out-DMA."""
                nfree = nr * WB
                sqv = s[(q, hh)].rearrange("p (h w) -> p h w", w=WP)
                sxv = (sx[(q, hh)].rearrange("p (h w) -> p h w", w=WP)
                       if shifted else None)
                ps_a = psum.tile([128, nfree], f32, tag="A", name=f"psa{q}{hh}{b}{r0}", bufs=3)
                ps_b = psum.tile([128, nfree], f32, tag="B", name=f"psb{q}{hh}{b}{r0}", bufs=3)
                for t in range(9):
                    i, j = t // 3, t % 3
                    first, last = t == 0, t == 8
                    wcol = (b * 9 + t) * F
                    if j == 1 and shifted:
                        rhs_a = sxv[0:64, r0 + i:r0 + i + nr,
                                    b * WB:b * WB + WB]
                        rhs_b = sxv[64:128, r0 + i:r0 + i + nr,
                                    b * WB:b * WB + WB]
                    else:
                        rhs_a = sqv[0:64, r0 + i:r0 + i + nr,
                                    b * WB + j:b * WB + j + WB]
                        rhs_b = sqv[64:128, r0 + i:r0 + i + nr,
                                    b * WB + j:b * WB + j + WB]
                    nc.tensor.matmul(
                        ps_a[:], wt[0:64, wcol:wcol + F], rhs_a,
                        start=first, stop=last,
                    )
                    nc.tensor.matmul(
                        ps_b[:], wt[64:128, wcol:wcol + F], rhs_b,
                        start=first, stop=last,
                    )
                # fused quant-scale + bias eviction into bf16 staging, then
                # straight out. A-half on ACT+SP ring, B-half on DVE+ACT ring.
                ev_a = stage.tile([128, nfree], bf16, tag="stA", name=f"eva{q}{hh}{b}{r0}")
                ev_b = stage.tile([128, nfree], bf16, tag="stB", name=f"evb{q}{hh}{b}{r0}")
                nc.scalar.activation(
                    ev_a[:], ps_a[:],
                    mybir.ActivationFunctionType.Identity,
                    bias=bsc_t[:, 0:1], scale=bsc_t[:, 1:2],
                )
                nc.vector.tensor_scalar(
                    ev_b[:], ps_b[:], bsc_t[:, 1:2], bsc_t[:, 0:1],
                    mybir.AluOpType.mult, mybir.AluOpType.add,
                )
                lo = hh * NT + r0 * WB
                nc.sync.dma_start(y[2 * q, :, b, lo:lo + nfree], ev_a[:])
                nc.scalar.dma_start(y[2 * q + 1, :, b, lo:lo + nfree], ev_b[:])

            # Group order: b1hh0 before b0hh1 (wt_b1 arrives before s01 on
            # SP), then hh-interleaved so each new b-weight chunk and each
            # new strip is consumed at half rate. First group h-split for
            # the earliest possible start.
            order = [(0, 0, 0), (0, 0, 1), (0, 1, 0), (0, 1, 1),
                     (0, 0, 2), (0, 1, 2), (0, 0, 3), (0, 1, 3)]
            for b in range(SW):
                for hh in range(2):
                    order.append((1, hh, b))
            for gi, (q, hh, b) in enumerate(order):
                # shifted copies only for groups starting >=3us after the
                # worst-case arrival of their sx tile
                sh = (q == 1) or (hh == 1 and b >= 2)
                if gi == 0:
                    emit_tile(q, hh, b, 0, SUB)
                    emit_tile(q, hh, b, SUB, HH - SUB)
                else:
                    emit_tile(q, hh, b, 0, HH, shifted=sh)

    nc.finalize()
    return nc


def _prep_inputs(inputs, kernel, bias):
    import ml_dtypes

    # fake-quant: integer part exact in bf16, scale folded into eviction
    scale = float(np.max(np.abs(kernel)) / 7.0)
    w_int = np.round(kernel[0] / scale).astype(np.float32)  # [SW, 576, F]

    # weight layout: [128 partitions, SW*9*F]; partition p holds channel
    # c = p % 64 (duplicated across halves for the two PE row groups)
    # free index (b*9 + t)*F + f  ->  w_int[b, c*9 + t, f]
    w4 = w_int.reshape(SW, C, 9, F)                    # [b, c, t, f]
    wt_half = np.transpose(w4, (1, 0, 2, 3)).reshape(C, SW * 9 * F)
    wts_host = np.concatenate([wt_half, wt_half], axis=0)  # [128, SW*9*F]
    wts_host = wts_host.astype(ml_dtypes.bfloat16)

    # input: pad spatially, bf16, pack image pairs into 128 partitions
    xp = np.zeros((N, C, HP, WP), np.float32)
    xp[:, :, 1:-1, 1:-1] = inputs
    xp = xp.astype(ml_dtypes.bfloat16).reshape(N, C, HP * WP)

    bsc_host = np.zeros((128, 2), np.float32)
    bsc_host[:, 0] = np.ascontiguousarray(bias, np.float32)
    bsc_host[:, 1] = scale

    in_maps = []
    for core in range(N_CORES):
        base = core * IMGS_PER_CORE
        xin = np.empty((PAIRS, 128, HP * WP), ml_dtypes.bfloat16)
        for q in range(PAIRS):
            xin[q, 0:64] = xp[base + 2 * q]
            xin[q, 64:128] = xp[base + 2 * q + 1]
        in_maps.append({
            "xin": xin,
            "wts": wts_host,
            "bsc": bsc_host,
        })
    return in_maps


def kernel(inputs, kernel, bias, _trace=False):
    from concourse.bass_utils import run_bass_kernel_spmd

    inputs = np.asarray(inputs)
    kernel = np.asarray(kernel)
    bias = np.asarray(bias)

    if "nc" not in _COMPILED:
        _COMPILED["nc"] = _build_program()
    nc = _COMPILED["nc"]

    in_maps = _prep_inputs(inputs, kernel, bias)
    res = run_bass_kernel_spmd(
        nc, in_maps, list(range(N_CORES)), trace=_trace
    )
    out = np.empty((N, F, H, W), np.float32)
    for core in range(N_CORES):
        base = core * IMGS_PER_CORE
        # y is b-major: [img, f, b, h*WB+wb] -> [img, f, h, b*WB+wb]
        yc = res.results[core]["y"].astype(np.float32)
        yc = yc.reshape(IMGS_PER_CORE, F, SW, H, WB).transpose(0, 1, 3, 2, 4)
        out[base:base + IMGS_PER_CORE] = yc.reshape(IMGS_PER_CORE, F, H, W)
    if _trace:
        return out, res
    return out


# revision 56
# speedup vs baseline: 1.0389x; 1.0389x over previous
"""LocalConv2D (3x3, width split into 4 weight blocks, 4-bit fake-quant weights)
on 8 Trainium2 NeuronCores.

Strategy
--------
Data-parallel over batch: 32 images -> 4 per core, processed as 2 pairs.
Image A of a pair lives in SBUF partitions 0-63 (its 64 channels), image B
in partitions 64-127. The 3x3 conv is 9 shifted K=64 matmuls accumulated in
PSUM; A's matmuls run in PE row-group 0 and B's in row-group 64
(tile_position auto-derived). The two K=64 matmuls of a tap stream
CONCURRENTLY through the PE array (disjoint row groups), so a tap pair
costs ~N cycles total.

Trace-derived facts this version is built around (17 profiled iterations):
- The HAM clock un-throttles (1.2->2.4 GHz) only after one fully-busy
  4096-cycle window of FULL-ARRAY activity. K=64 single-row-group warmup
  does NOT count as busy; warmup here runs as row-group PAIRS from the
  earliest possible instant (~7.5us) and is sized to bridge seamlessly to
  the first real matmul (~12.5us) — a PE idle >=1.5us there re-throttles
  the clock for ~3.4us (measured repeatedly).
- An A/B pair's PSUM banks must not coincide mod 4 or the concurrent
  drains serialize (~+35ns/pair); tag layout keeps them 3 apart.
- DMA queue first-data latency: SP ~1.5us stable; ACT 1.5-3.0us run-
  variable. A DMA's completion semaphore lags its data by ~0.4us at queue
  depth 1 growing to 2-5us at depth 3+ (high variance). Hence: coarse
  whole-tensor transfers; everything PE-gating at depth <=2 (q0 strips
  lead SP, b0 weights lead ACT as ONE chunk); depth >=3 slots hold only
  items with >=5us slack (b1 weights, bias, q1 strips, b2/b3 weights).
- Group order interleaves hh within each b so each arriving strip/weight
  chunk is consumed at half rate, doubling every DMA-arrival margin. The
  first group is h-split into two 14-row tiles for the earliest start.
- Warm steady state measures ~164ns per tap pair = the N/2.4GHz stream
  floor. The graded window also includes a fixed ~8.5us walrus epilogue
  (semaphore resets) that no kernel structure can avoid.

Weights are fake-quantized per-tensor to 4 bits: q = round(w/s)*s with
s = max|w|/7. round(w/s) is a small integer in [-7,7], exactly
representable in bf16, so the matmul runs on exact integer weights and the
scale s is folded into the eviction (out = psum*s + bias). Output staged
bf16 (fro-error ~2.3e-3, well under the 2e-2 gate).
"""

import numpy as np

KSIZE = 3
SW = 4
KBITS = 4
N, C, H, W, F = 32, 64, 56, 56, 128
HP, WP = H + 2, W + 2          # padded 58x58
N_CORES = 8
IMGS_PER_CORE = N // N_CORES   # 4
PAIRS = IMGS_PER_CORE // 2     # 2
WB = W // SW                   # 14
HH = H // 2                    # 28 out rows per h-half tile
HR = HH + 2                    # 30 input rows feeding one h-half
SUB = HH // 2                  # 14 out rows per h-quarter sub-tile
SUBR = SUB + 2                 # 16 input rows feeding a sub-tile

NT = HH * WB                   # 392 free elems per full tile
NS = SUB * WB                  # 196 free elems per sub tile

W_N = 392                      # warmup moving size (cold pair ~327ns)
W_SHORT = 3                    # short (N=128) warmup pairs while memset part 2 lands
W_LONG = 16                    # long (N=392) warmup pairs

_COMPILED = {}


def _install_drain_patch():
    """The walrus build here rejects instructions carrying >2 sync waits
    ('Too many sync wait commands'). Two fixes, both relying on engines
    executing their own stream in order:

    1. _add_instruction: any scheduled instruction with >2 waits gets
       same-engine NoOps inserted before it, each carrying <=2 of the waits.
    2. The Tile tail drain gets one wait per outstanding logical proc; emit
       one SP nop per proc, then strip the duplicated waits off the drain.
    """
    import re
    import bass_rust
    from concourse.vector_clock import ScopedClock
    import concourse.tile as tile
    import concourse.mybir as mybir

    if getattr(tile.TileContext, "_drain_patch_installed", False):
        return

    MAXW = 1       # this walrus build fits exactly 1 sync wait per instruction
    NOP_MAXW = 1
    _orig_add = tile.TileContext._add_instruction

    def _add_split(self, inst):
        si = getattr(inst, "sync_info", None)
        if si is not None and si.on_wait and len(si.on_wait) > MAXW:
            waits = list(si.on_wait)
            while len(waits) > MAXW:
                chunk, waits = waits[:NOP_MAXW], waits[NOP_MAXW:]
                nop = mybir.InstNoOp(
                    name=self.nc.get_next_instruction_name(), ins=[], outs=[]
                )
                nop.engine = inst.engine
                nop.sync_info = bass_rust.SyncInfo(on_wait=chunk, on_update=[])
                _orig_add(self, nop)
            si.on_wait = waits
        return _orig_add(self, inst)

    tile.TileContext._add_instruction = _add_split

    _orig = tile.TileContext._drain_and_barrier

    def _split(self, tick_clock, wait_clock):
        gc = tick_clock.global_clock
        m = re.match(r"VectorClock\(\[(.*)\]\)", repr(gc))
        vals = [int(v) for v in m.group(1).split(",")] if m.group(1).strip() else []
        covered = set()
        # Round-robin the per-proc wait nops across engines so they wait in
        # parallel (serial on one engine costs ~55ns each x ~30 procs). The
        # all-engine barrier that _orig emits right after collects them.
        engines = [
            self.nc.sync, self.nc.scalar, self.nc.vector,
            self.nc.gpsimd, self.nc.tensor,
        ]
        eng_i = 0
        for i, v in enumerate(vals):
            if v == 0:
                continue
            part = [0] * len(vals)
            part[i] = v
            nop = engines[eng_i % len(engines)].nop()
            eng_i += 1
            wait_clock.add_sem_waits(
                nop.ins, ScopedClock({None: bass_rust.VectorClock(part)})
            )
            si = nop.ins.sync_info
            for w in (si.on_wait if si else []) or []:
                covered.add((w.ant_name, w.wait_value))
        holder = []
        orig_drain = self.nc.sync.drain

        def capture(*a, **k):
            inst = orig_drain(*a, **k)
            holder.append(inst)
            return inst

        self.nc.sync.drain = capture
        try:
            r = _orig(self, tick_clock, wait_clock)
        finally:
            self.nc.sync.drain = orig_drain
        if holder:
            inst = holder[0].ins if hasattr(holder[0], "ins") else holder[0]
            si = inst.sync_info
            if si and si.on_wait:
                si.on_wait = [
                    w for w in si.on_wait
                    if (w.ant_name, w.wait_value) not in covered
                ]
        return r

    tile.TileContext._drain_and_barrier = _split
    tile.TileContext._drain_patch_installed = True


def _build_program():
    import concourse.bass as bass
    import concourse.mybir as mybir
    import concourse.tile as tile

    _install_drain_patch()

    f32 = mybir.dt.float32
    bf16 = mybir.dt.bfloat16

    nc = bass.Bass(target_bir_lowering=False, debug=False)
    # per-core inputs: image pairs packed [128, HP*WP] (A in 0-63, B 64-127)
    xin = nc.declare_dram_parameter(
        "xin", [PAIRS, 128, HP * WP], bf16, isOutput=False
    )
    # weights duplicated across halves: col (b*9 + t)*F + f
    wts = nc.declare_dram_parameter("wts", [128, SW * 9 * F], bf16, isOutput=False)
    bsc_p = nc.declare_dram_parameter("bsc", [128, 2], f32, isOutput=False)
    # b-major output layout: block (b, hh) of an image is contiguous, so
    # each PSUM eviction can stream straight to DRAM. Host reassembles.
    y = nc.declare_dram_parameter(
        "y", [IMGS_PER_CORE, 128, SW, H * WB], bf16, isOutput=True
    )

    WCH = 9 * F                  # weight columns per b-block

    with tile.TileContext(nc) as tc:
        with (
            tc.tile_pool(name="consts", bufs=1) as consts,
            tc.tile_pool(name="strips", bufs=1) as strips,
            tc.tile_pool(name="stage", bufs=8) as stage,
            tc.tile_pool(name="psum", bufs=4, space="PSUM") as psum,
        ):
            # Dummy source for warmup matmuls. Memset on GpSimd (otherwise
            # idle) in two stages: a small first chunk so the first LDW can
            # issue ~0.3us after the preamble barrier, the rest behind it.
            warm = consts.tile([128, W_N], bf16)
            nc.gpsimd.memset(warm[:, 0:F], 0.0)
            nc.gpsimd.memset(warm[:, F:W_N], 0.0)

            wt = consts.tile([128, SW * 9 * F], bf16)
            bsc_t = consts.tile([128, 2], f32)

            # --- input DMA schedule (order = queue order) ---
            # All of b0's weights must be resident before the first real
            # matmul: a mid-group weight wait stalls the PE long enough to
            # re-throttle the HAM clock (measured -6us). Split b0 across
            # both queues so the slower ACT ring doesn't gate, and lead SP
            # with taps 0-4 ahead of the strips.
            s = {}
            for q in range(PAIRS):
                for hh in range(2):
                    st = strips.tile([128, HR * WP], bf16, tag=f"s{q}{hh}")
                    s[(q, hh)] = st
            # Coarse whole-tensor transfers: the queues' data rate degrades
            # and completion semaphores lag by 1-3us when many DMAs are
            # queued, so few big DMAs with wide consumption margins beat
            # fine-grained racing (measured across three schedule attempts).
            # SP ring (fast: ~1.4us first-data, lower completion-sem lag):
            # the two deadline-critical q0 strips, then b2/b3 weights.
            # ACT ring (slow: ~3us first-data, sems lag up to ~2us): tiny
            # bias/scale first, b0 weights in two chunks (first chunk's sem
            # unblocks the first matmul ~1us sooner than one 295KB DMA),
            # b1 weights, then the q1 strips whose deadlines are ~12us out.
            # Outputs ride the same two rings; the deep `stage` pool
            # absorbs the out-DMA backlog while inputs still stream.
            # Queue facts (measured over ten runs): first-data latency is
            # ~1.5us on SP but 1.5-3.0us run-variable on ACT; a DMA's
            # completion semaphore lags its data by ~0.4us at queue depth 1
            # growing to ~2us at depth 3+. So: the first matmul's needs are
            # split across BOTH queue heads (b0 taps 0-4 lead SP, taps 5-8
            # lead ACT), strips follow on SP, and everything at depth >=3
            # on either queue has a deadline >=4us out.
            # Semaphore-lag law (measured): depth-1 sems fire ~0.4us after
            # data, depth-2 ~1us, depth-3+ can lag 2-4us on EITHER queue.
            # So only two tight-deadline items exist per queue (depths
            # 1-2); the group order below (all hh0 first) pushes every
            # other consumer's deadline past +5us.
            # SP: s00 (gates first matmul), wt_b1 (deadline +1.7us),
            #     bias/scale (soft deadline - PSUM/stage cushion), q1
            #     strips (deadline +13us).
            nc.sync.dma_start(s[(0, 0)][:], xin[0, :, 0:HR * WP])
            nc.sync.dma_start(wt[:, WCH:2 * WCH], wts[:, WCH:2 * WCH])
            nc.sync.dma_start(bsc_t[:], bsc_p[:])
            nc.sync.dma_start(s[(1, 0)][:], xin[1, :, 0:HR * WP])
            nc.sync.dma_start(
                s[(1, 1)][:], xin[1, :, HH * WP:(HH + HR) * WP]
            )
            # ACT: wt_b0 as ONE chunk (gates first matmul; a split chunk
            # puts taps 5-8 at depth 2 whose sem can stall mid-group),
            # wt_b2 (+3.4us), wt_b3 (+5.1us), s01 (+6.8us), then the
            # shifted copies.
            nc.scalar.dma_start(wt[:, 0:WCH], wts[:, 0:WCH])
            nc.scalar.dma_start(wt[:, 2 * WCH:3 * WCH], wts[:, 2 * WCH:3 * WCH])
            nc.scalar.dma_start(wt[:, 3 * WCH:4 * WCH], wts[:, 3 * WCH:4 * WCH])
            nc.scalar.dma_start(
                s[(0, 1)][:], xin[0, :, HH * WP:(HH + HR) * WP]
            )
            # Shifted strip copies for aligned j=1 taps: an odd-element
            # moving base costs +23ns per matmul pair (a third of all
            # taps). The shifted copy is the SAME DRAM at offset +1 element
            # DMA'd into an even-aligned SBUF tile — no host prep, no
            # engine copies. They ride the otherwise-idle ACT tail and are
            # consumed only by groups whose start is >=3us past the
            # worst-case arrival (late q0-hh1 groups and all of q1).
            sx = {}
            for (q, hh) in [(0, 1), (1, 0), (1, 1)]:
                st = strips.tile(
                    [128, HR * WP], bf16, tag=f"x{q}{hh}", name=f"sx{q}{hh}"
                )
                sx[(q, hh)] = st
                off = hh * HH * WP
                nc.scalar.dma_start(
                    st[:, 0:HR * WP - 1],
                    xin[q, :, off + 1:off + HR * WP],
                )

            # --- PE warmup: full-array paired matmuls, no DMA dependency ---
            # PSUM slot layout note: tag "W" occupies banks 0-1, "A" banks
            # 2-4, "B" banks 5-7. An A/B pair is always 3 banks apart —
            # pairs whose banks coincide mod 4 measurably serialize their
            # concurrent drains (~+35ns/pair warm), so keep the distance 3.
            # One long accumulation chain per bank: no inter-group WAW
            # stalls. First pairs are short (N=128) so they only need the
            # first memset chunk.
            wa = psum.tile([128, W_N], f32, tag="W", name="wa", bufs=2)
            wb = psum.tile([128, W_N], f32, tag="W", name="wb", bufs=2)
            for k in range(W_SHORT + W_LONG):
                first, last = k == 0, k == W_SHORT + W_LONG - 1
                n = F if k < W_SHORT else W_N
                nc.tensor.matmul(
                    wa[:, 0:n], warm[0:64, 0:F], warm[0:64, 0:n],
                    start=first, stop=last,
                )
                nc.tensor.matmul(
                    wb[:, 0:n], warm[64:128, 0:F], warm[64:128, 0:n],
                    start=first, stop=last,
                )

            def emit_tile(q, hh, b, r0, nr, shifted=False):
                """One output tile: image pair q, h-half hh, width block b,
                out rows [r0, r0+nr) of the half. Returns nothing; emits
                matmuls + eviction + out-DMA."""
                nfree = nr * WB
                sqv = s[(q, hh)].rearrange("p (h w) -> p h w", w=WP)
                sxv = (sx[(q, hh)].rearrange("p (h w) -> p h w", w=WP)
                       if shifted else None)
                ps_a = psum.tile([128, nfree], f32, tag="A", name=f"psa{q}{hh}{b}{r0}", bufs=3)
                ps_b = psum.tile([128, nfree], f32, tag="B", name=f"psb{q}{hh}{b}{r0}", bufs=3)
                for t in range(9):
                    i, j = t // 3, t % 3
                    first, last = t == 0, t == 8
                    wcol = (b * 9 + t) * F
                    if j == 1 and shifted:
                        rhs_a = sxv[0:64, r0 + i:r0 + i + nr,
                                    b * WB:b * WB + WB]
                        rhs_b = sxv[64:128, r0 + i:r0 + i + nr,
                                    b * WB:b * WB + WB]
                    else:
                        rhs_a = sqv[0:64, r0 + i:r0 + i + nr,
                                    b * WB + j:b * WB + j + WB]
                        rhs_b = sqv[64:128, r0 + i:r0 + i + nr,
                                    b * WB + j:b * WB + j + WB]
                    nc.tensor.matmul(
                        ps_a[:], wt[0:64, wcol:wcol + F], rhs_a,
                        start=first, stop=last,
                    )
                    nc.tensor.matmul(
                        ps_b[:], wt[64:128, wcol:wcol + F], rhs_b,
                        start=first, stop=last,
                    )
                # fused quant-scale + bias eviction into bf16 staging, then
                # straight out. A-half on ACT+SP ring, B-half on DVE+ACT ring.
                ev_a = stage.tile([128, nfree], bf16, tag="stA", name=f"eva{q}{hh}{b}{r0}")
                ev_b = stage.tile([128, nfree], bf16, tag="stB", name=f"evb{q}{hh}{b}{r0}")
                nc.scalar.activation(
                    ev_a[:], ps_a[:],
                    mybir.ActivationFunctionType.Identity,
                    bias=bsc_t[:, 0:1], scale=bsc_t[:, 1:2],
                )
                nc.vector.tensor_scalar(
                    ev_b[:], ps_b[:], bsc_t[:, 1:2], bsc_t[:, 0:1],
                    mybir.AluOpType.mult, mybir.AluOpType.add,
                )
                lo = hh * NT + r0 * WB
                nc.sync.dma_start(y[2 * q, :, b, lo:lo + nfree], ev_a[:])
                nc.scalar.dma_start(y[2 * q + 1, :, b, lo:lo + nfree], ev_b[:])

            # Group order: all hh0 groups first (only s00 + weights
            # needed), then the hh1 pass — s01's deadline lands +6.8us
            # after the first matmul, safely past any sem-lag draw. Same
            # shape for q1. First group h-split for the earliest start.
            order = []
            for q in range(PAIRS):
                for hh in range(2):
                    for b in range(SW):
                        order.append((q, hh, b))
            for gi, (q, hh, b) in enumerate(order):
                # shifted copies only for groups starting >=3us after the
                # worst-case arrival of their sx tile
                sh = (q == 1) or (hh == 1 and b >= 2)
                if gi == 0:
                    emit_tile(q, hh, b, 0, SUB)
                    emit_tile(q, hh, b, SUB, HH - SUB)
                else:
                    emit_tile(q, hh, b, 0, HH, shifted=sh)

    nc.finalize()
    return nc


def _prep_inputs(inputs, kernel, bias):
    import ml_dtypes

    # fake-quant: integer part exact in bf16, scale folded into eviction
    scale = float(np.max(np.abs(kernel)) / 7.0)
    w_int = np.round(kernel[0] / scale).astype(np.float32)  # [SW, 576, F]

    # weight layout: [128 partitions, SW*9*F]; partition p holds channel
    # c = p % 64 (duplicated across halves for the two PE row groups)
    # free index (b*9 + t)*F + f  ->  w_int[b, c*9 + t, f]
    w4 = w_int.reshape(SW, C, 9, F)                    # [b, c, t, f]
    wt_half = np.transpose(w4, (1, 0, 2, 3)).reshape(C, SW * 9 * F)
    wts_host = np.concatenate([wt_half, wt_half], axis=0)  # [128, SW*9*F]
    wts_host = wts_host.astype(ml_dtypes.bfloat16)

    # input: pad spatially, bf16, pack image pairs into 128 partitions
    xp = np.zeros((N, C, HP, WP), np.float32)
    xp[:, :, 1:-1, 1:-1] = inputs
    xp = xp.astype(ml_dtypes.bfloat16).reshape(N, C, HP * WP)

    bsc_host = np.zeros((128, 2), np.float32)
    bsc_host[:, 0] = np.ascontiguousarray(bias, np.float32)
    bsc_host[:, 1] = scale

    in_maps = []
    for core in range(N_CORES):
        base = core * IMGS_PER_CORE
        xin = np.empty((PAIRS, 128, HP * WP), ml_dtypes.bfloat16)
        for q in range(PAIRS):
            xin[q, 0:64] = xp[base + 2 * q]
            xin[q, 64:128] = xp[base + 2 * q + 1]
        in_maps.append({
            "xin": xin,
            "wts": wts_host,
            "bsc": bsc_host,
        })
    return in_maps


def kernel(inputs, kernel, bias, _trace=False):
    from concourse.bass_utils import run_bass_kernel_spmd

    inputs = np.asarray(inputs)
    kernel = np.asarray(kernel)
    bias = np.asarray(bias)

    if "nc" not in _COMPILED:
        _COMPILED["nc"] = _build_program()
    nc = _COMPILED["nc"]

    in_maps = _prep_inputs(inputs, kernel, bias)
    res = run_bass_kernel_spmd(
        nc, in_maps, list(range(N_CORES)), trace=_trace
    )
    out = np.empty((N, F, H, W), np.float32)
    for core in range(N_CORES):
        base = core * IMGS_PER_CORE
        # y is b-major: [img, f, b, h*WB+wb] -> [img, f, h, b*WB+wb]
        yc = res.results[core]["y"].astype(np.float32)
        yc = yc.reshape(IMGS_PER_CORE, F, SW, H, WB).transpose(0, 1, 3, 2, 4)
        out[base:base + IMGS_PER_CORE] = yc.reshape(IMGS_PER_CORE, F, H, W)
    if _trace:
        return out, res
    return out


# revision 59
# speedup vs baseline: 1.0553x; 1.0158x over previous
"""LocalConv2D (3x3, width split into 4 weight blocks, 4-bit fake-quant weights)
on 8 Trainium2 NeuronCores.

Strategy
--------
Data-parallel over batch: 32 images -> 4 per core, processed as 2 pairs.
Image A of a pair lives in SBUF partitions 0-63 (its 64 channels), image B
in partitions 64-127. The 3x3 conv is 9 shifted K=64 matmuls accumulated in
PSUM; A's matmuls run in PE row-group 0 and B's in row-group 64
(tile_position auto-derived). The two K=64 matmuls of a tap stream
CONCURRENTLY through the PE array (disjoint row groups), so a tap pair
costs ~N cycles total.

Trace-derived facts this version is built around (17 profiled iterations):
- The HAM clock un-throttles (1.2->2.4 GHz) only after one fully-busy
  4096-cycle window of FULL-ARRAY activity. K=64 single-row-group warmup
  does NOT count as busy; warmup here runs as row-group PAIRS from the
  earliest possible instant (~7.5us) and is sized to bridge seamlessly to
  the first real matmul (~12.5us) — a PE idle >=1.5us there re-throttles
  the clock for ~3.4us (measured repeatedly).
- An A/B pair's PSUM banks must not coincide mod 4 or the concurrent
  drains serialize (~+35ns/pair); tag layout keeps them 3 apart.
- DMA queue first-data latency: SP ~1.5us stable; ACT 1.5-3.0us run-
  variable. A DMA's completion semaphore lags its data by ~0.4us at queue
  depth 1 growing to 2-5us at depth 3+ (high variance). Hence: coarse
  whole-tensor transfers; everything PE-gating at depth <=2 (q0 strips
  lead SP, b0 weights lead ACT as ONE chunk); depth >=3 slots hold only
  items with >=5us slack (b1 weights, bias, q1 strips, b2/b3 weights).
- Group order interleaves hh within each b so each arriving strip/weight
  chunk is consumed at half rate, doubling every DMA-arrival margin. The
  first group is h-split into two 14-row tiles for the earliest start.
- Warm steady state measures ~164ns per tap pair = the N/2.4GHz stream
  floor. The graded window also includes a fixed ~8.5us walrus epilogue
  (semaphore resets) that no kernel structure can avoid.

Weights are fake-quantized per-tensor to 4 bits: q = round(w/s)*s with
s = max|w|/7. round(w/s) is a small integer in [-7,7], exactly
representable in bf16, so the matmul runs on exact integer weights and the
scale s is folded into the eviction (out = psum*s + bias). Output staged
bf16 (fro-error ~2.3e-3, well under the 2e-2 gate).
"""

import numpy as np

KSIZE = 3
SW = 4
KBITS = 4
N, C, H, W, F = 32, 64, 56, 56, 128
HP, WP = H + 2, W + 2          # padded 58x58
N_CORES = 8
IMGS_PER_CORE = N // N_CORES   # 4
PAIRS = IMGS_PER_CORE // 2     # 2
WB = W // SW                   # 14
HH = H // 2                    # 28 out rows per h-half tile
HR = HH + 2                    # 30 input rows feeding one h-half
SUB = HH // 2                  # 14 out rows per h-quarter sub-tile
SUBR = SUB + 2                 # 16 input rows feeding a sub-tile

NT = HH * WB                   # 392 free elems per full tile
NS = SUB * WB                  # 196 free elems per sub tile

W_N = 392                      # warmup moving size (cold pair ~327ns)
W_SHORT = 3                    # short (N=128) warmup pairs while memset part 2 lands
W_LONG = 16                    # long (N=392) warmup pairs

_COMPILED = {}


def _install_drain_patch():
    """The walrus build here rejects instructions carrying >2 sync waits
    ('Too many sync wait commands'). Two fixes, both relying on engines
    executing their own stream in order:

    1. _add_instruction: any scheduled instruction with >2 waits gets
       same-engine NoOps inserted before it, each carrying <=2 of the waits.
    2. The Tile tail drain gets one wait per outstanding logical proc; emit
       one SP nop per proc, then strip the duplicated waits off the drain.
    """
    import re
    import bass_rust
    from concourse.vector_clock import ScopedClock
    import concourse.tile as tile
    import concourse.mybir as mybir

    if getattr(tile.TileContext, "_drain_patch_installed", False):
        return

    MAXW = 1       # this walrus build fits exactly 1 sync wait per instruction
    NOP_MAXW = 1
    _orig_add = tile.TileContext._add_instruction

    def _add_split(self, inst):
        si = getattr(inst, "sync_info", None)
        if si is not None and si.on_wait and len(si.on_wait) > MAXW:
            waits = list(si.on_wait)
            while len(waits) > MAXW:
                chunk, waits = waits[:NOP_MAXW], waits[NOP_MAXW:]
                nop = mybir.InstNoOp(
                    name=self.nc.get_next_instruction_name(), ins=[], outs=[]
                )
                nop.engine = inst.engine
                nop.sync_info = bass_rust.SyncInfo(on_wait=chunk, on_update=[])
                _orig_add(self, nop)
            si.on_wait = waits
        return _orig_add(self, inst)

    tile.TileContext._add_instruction = _add_split

    _orig = tile.TileContext._drain_and_barrier

    def _split(self, tick_clock, wait_clock):
        gc = tick_clock.global_clock
        m = re.match(r"VectorClock\(\[(.*)\]\)", repr(gc))
        vals = [int(v) for v in m.group(1).split(",")] if m.group(1).strip() else []
        covered = set()
        # Round-robin the per-proc wait nops across engines so they wait in
        # parallel (serial on one engine costs ~55ns each x ~30 procs). The
        # all-engine barrier that _orig emits right after collects them.
        engines = [
            self.nc.sync, self.nc.scalar, self.nc.vector,
            self.nc.gpsimd, self.nc.tensor,
        ]
        eng_i = 0
        for i, v in enumerate(vals):
            if v == 0:
                continue
            part = [0] * len(vals)
            part[i] = v
            nop = engines[eng_i % len(engines)].nop()
            eng_i += 1
            wait_clock.add_sem_waits(
                nop.ins, ScopedClock({None: bass_rust.VectorClock(part)})
            )
            si = nop.ins.sync_info
            for w in (si.on_wait if si else []) or []:
                covered.add((w.ant_name, w.wait_value))
        holder = []
        orig_drain = self.nc.sync.drain

        def capture(*a, **k):
            inst = orig_drain(*a, **k)
            holder.append(inst)
            return inst

        self.nc.sync.drain = capture
        try:
            r = _orig(self, tick_clock, wait_clock)
        finally:
            self.nc.sync.drain = orig_drain
        if holder:
            inst = holder[0].ins if hasattr(holder[0], "ins") else holder[0]
            si = inst.sync_info
            if si and si.on_wait:
                si.on_wait = [
                    w for w in si.on_wait
                    if (w.ant_name, w.wait_value) not in covered
                ]
        return r

    tile.TileContext._drain_and_barrier = _split
    tile.TileContext._drain_patch_installed = True


def _build_program():
    import concourse.bass as bass
    import concourse.mybir as mybir
    import concourse.tile as tile

    _install_drain_patch()

    f32 = mybir.dt.float32
    bf16 = mybir.dt.bfloat16

    nc = bass.Bass(target_bir_lowering=False, debug=False)
    # per-core inputs: image pairs packed [128, HP*WP] (A in 0-63, B 64-127)
    xin = nc.declare_dram_parameter(
        "xin", [PAIRS, 128, HP * WP], bf16, isOutput=False
    )
    # weights duplicated across halves: col (b*9 + t)*F + f
    wts = nc.declare_dram_parameter("wts", [128, SW * 9 * F], bf16, isOutput=False)
    bsc_p = nc.declare_dram_parameter("bsc", [128, 2], f32, isOutput=False)
    # b-major output layout: block (b, hh) of an image is contiguous, so
    # each PSUM eviction can stream straight to DRAM. Host reassembles.
    y = nc.declare_dram_parameter(
        "y", [IMGS_PER_CORE, 128, SW, H * WB], bf16, isOutput=True
    )

    WCH = 9 * F                  # weight columns per b-block

    with tile.TileContext(nc) as tc:
        with (
            tc.tile_pool(name="consts", bufs=1) as consts,
            tc.tile_pool(name="strips", bufs=1) as strips,
            tc.tile_pool(name="stage", bufs=8) as stage,
            tc.tile_pool(name="psum", bufs=4, space="PSUM") as psum,
        ):
            # Dummy source for warmup matmuls. Memset on GpSimd (otherwise
            # idle) in two stages: a small first chunk so the first LDW can
            # issue ~0.3us after the preamble barrier, the rest behind it.
            warm = consts.tile([128, W_N], bf16)
            nc.gpsimd.memset(warm[:, 0:F], 0.0)
            nc.gpsimd.memset(warm[:, F:W_N], 0.0)

            wt = consts.tile([128, SW * 9 * F], bf16)
            bsc_t = consts.tile([128, 2], f32)

            # --- input DMA schedule (order = queue order) ---
            # All of b0's weights must be resident before the first real
            # matmul: a mid-group weight wait stalls the PE long enough to
            # re-throttle the HAM clock (measured -6us). Split b0 across
            # both queues so the slower ACT ring doesn't gate, and lead SP
            # with taps 0-4 ahead of the strips.
            s = {}
            for q in range(PAIRS):
                for hh in range(2):
                    st = strips.tile([128, HR * WP], bf16, tag=f"s{q}{hh}")
                    s[(q, hh)] = st
            # Coarse whole-tensor transfers: the queues' data rate degrades
            # and completion semaphores lag by 1-3us when many DMAs are
            # queued, so few big DMAs with wide consumption margins beat
            # fine-grained racing (measured across three schedule attempts).
            # SP ring (fast: ~1.4us first-data, lower completion-sem lag):
            # the two deadline-critical q0 strips, then b2/b3 weights.
            # ACT ring (slow: ~3us first-data, sems lag up to ~2us): tiny
            # bias/scale first, b0 weights in two chunks (first chunk's sem
            # unblocks the first matmul ~1us sooner than one 295KB DMA),
            # b1 weights, then the q1 strips whose deadlines are ~12us out.
            # Outputs ride the same two rings; the deep `stage` pool
            # absorbs the out-DMA backlog while inputs still stream.
            # Queue facts (measured over ten runs): first-data latency is
            # ~1.5us on SP but 1.5-3.0us run-variable on ACT; a DMA's
            # completion semaphore lags its data by ~0.4us at queue depth 1
            # growing to ~2us at depth 3+. So: the first matmul's needs are
            # split across BOTH queue heads (b0 taps 0-4 lead SP, taps 5-8
            # lead ACT), strips follow on SP, and everything at depth >=3
            # on either queue has a deadline >=4us out.
            # Semaphore-lag law (measured): depth-1 sems fire ~0.4us after
            # data, depth-2 ~1us, depth-3+ can lag 2-4us on EITHER queue.
            # So only two tight-deadline items exist per queue (depths
            # 1-2); the group order below (all hh0 first) pushes every
            # other consumer's deadline past +5us.
            # SP: s00 (gates first matmul), wt_b1 (deadline +1.7us),
            #     bias/scale (soft deadline - PSUM/stage cushion), q1
            #     strips (deadline +13us).
            nc.sync.dma_start(s[(0, 0)][:], xin[0, :, 0:HR * WP])
            nc.sync.dma_start(wt[:, WCH:2 * WCH], wts[:, WCH:2 * WCH])
            nc.sync.dma_start(
                s[(0, 1)][:], xin[0, :, HH * WP:(HH + HR) * WP]
            )
            nc.sync.dma_start(bsc_t[:], bsc_p[:])
            nc.sync.dma_start(s[(1, 0)][:], xin[1, :, 0:HR * WP])
            nc.sync.dma_start(
                s[(1, 1)][:], xin[1, :, HH * WP:(HH + HR) * WP]
            )
            # ACT: wt_b0 as ONE chunk (gates first matmul; a split chunk
            # puts taps 5-8 at depth 2 whose sem can stall mid-group),
            # wt_b2 (+3.4us), wt_b3 (+5.1us).
            nc.scalar.dma_start(wt[:, 0:WCH], wts[:, 0:WCH])
            nc.scalar.dma_start(wt[:, 2 * WCH:3 * WCH], wts[:, 2 * WCH:3 * WCH])
            nc.scalar.dma_start(wt[:, 3 * WCH:4 * WCH], wts[:, 3 * WCH:4 * WCH])
            # Shifted strip copies for aligned j=1 taps: an odd-element
            # moving base costs +23ns per matmul pair (a third of all
            # taps). The shifted copy is the SAME DRAM at offset +1 element
            # DMA'd into an even-aligned SBUF tile — no host prep, no
            # engine copies. They ride the otherwise-idle ACT tail and are
            # consumed only by groups whose start is >=3us past the
            # worst-case arrival (late q0-hh1 groups and all of q1).
            sx = {}
            for (q, hh) in [(1, 0), (1, 1)]:
                st = strips.tile(
                    [128, HR * WP], bf16, tag=f"x{q}{hh}", name=f"sx{q}{hh}"
                )
                sx[(q, hh)] = st
                off = hh * HH * WP
                nc.scalar.dma_start(
                    st[:, 0:HR * WP - 1],
                    xin[q, :, off + 1:off + HR * WP],
                )

            # --- PE warmup: full-array paired matmuls, no DMA dependency ---
            # PSUM slot layout note: tag "W" occupies banks 0-1, "A" banks
            # 2-4, "B" banks 5-7. An A/B pair is always 3 banks apart —
            # pairs whose banks coincide mod 4 measurably serialize their
            # concurrent drains (~+35ns/pair warm), so keep the distance 3.
            # One long accumulation chain per bank: no inter-group WAW
            # stalls. First pairs are short (N=128) so they only need the
            # first memset chunk.
            wa = psum.tile([128, W_N], f32, tag="W", name="wa", bufs=2)
            wb = psum.tile([128, W_N], f32, tag="W", name="wb", bufs=2)
            for k in range(W_SHORT + W_LONG):
                first, last = k == 0, k == W_SHORT + W_LONG - 1
                n = F if k < W_SHORT else W_N
                nc.tensor.matmul(
                    wa[:, 0:n], warm[0:64, 0:F], warm[0:64, 0:n],
                    start=first, stop=last,
                )
                nc.tensor.matmul(
                    wb[:, 0:n], warm[64:128, 0:F], warm[64:128, 0:n],
                    start=first, stop=last,
                )

            def emit_tile(q, hh, b, r0, nr, shifted=False):
                """One output tile: image pair q, h-half hh, width block b,
                out rows [r0, r0+nr) of the half. Returns nothing; emits
                matmuls + eviction + out-DMA."""
                nfree = nr * WB
                sqv = s[(q, hh)].rearrange("p (h w) -> p h w", w=WP)
                sxv = (sx[(q, hh)].rearrange("p (h w) -> p h w", w=WP)
                       if shifted else None)
                ps_a = psum.tile([128, nfree], f32, tag="A", name=f"psa{q}{hh}{b}{r0}", bufs=3)
                ps_b = psum.tile([128, nfree], f32, tag="B", name=f"psb{q}{hh}{b}{r0}", bufs=3)
                for t in range(9):
                    i, j = t // 3, t % 3
                    first, last = t == 0, t == 8
                    wcol = (b * 9 + t) * F
                    if j == 1 and shifted:
                        rhs_a = sxv[0:64, r0 + i:r0 + i + nr,
                                    b * WB:b * WB + WB]
                        rhs_b = sxv[64:128, r0 + i:r0 + i + nr,
                                    b * WB:b * WB + WB]
                    else:
                        rhs_a = sqv[0:64, r0 + i:r0 + i + nr,
                                    b * WB + j:b * WB + j + WB]
                        rhs_b = sqv[64:128, r0 + i:r0 + i + nr,
                                    b * WB + j:b * WB + j + WB]
                    nc.tensor.matmul(
                        ps_a[:], wt[0:64, wcol:wcol + F], rhs_a,
                        start=first, stop=last,
                    )
                    nc.tensor.matmul(
                        ps_b[:], wt[64:128, wcol:wcol + F], rhs_b,
                        start=first, stop=last,
                    )
                # fused quant-scale + bias eviction into bf16 staging, then
                # straight out. A-half on ACT+SP ring, B-half on DVE+ACT ring.
                ev_a = stage.tile([128, nfree], bf16, tag="stA", name=f"eva{q}{hh}{b}{r0}")
                ev_b = stage.tile([128, nfree], bf16, tag="stB", name=f"evb{q}{hh}{b}{r0}")
                nc.scalar.activation(
                    ev_a[:], ps_a[:],
                    mybir.ActivationFunctionType.Identity,
                    bias=bsc_t[:, 0:1], scale=bsc_t[:, 1:2],
                )
                nc.vector.tensor_scalar(
                    ev_b[:], ps_b[:], bsc_t[:, 1:2], bsc_t[:, 0:1],
                    mybir.AluOpType.mult, mybir.AluOpType.add,
                )
                lo = hh * NT + r0 * WB
                nc.sync.dma_start(y[2 * q, :, b, lo:lo + nfree], ev_a[:])
                nc.scalar.dma_start(y[2 * q + 1, :, b, lo:lo + nfree], ev_b[:])

            # Group order: all hh0 groups first (only s00 + weights
            # needed), then the hh1 pass — s01's deadline lands +6.8us
            # after the first matmul, safely past any sem-lag draw. Same
            # shape for q1. First group h-split for the earliest start.
            order = []
            for q in range(PAIRS):
                for hh in range(2):
                    for b in range(SW):
                        order.append((q, hh, b))
            for gi, (q, hh, b) in enumerate(order):
                # shifted copies only for q1: those groups start >=5us
                # after the worst-case arrival of their sx tile
                sh = q == 1
                if gi == 0:
                    emit_tile(q, hh, b, 0, SUB)
                    emit_tile(q, hh, b, SUB, HH - SUB)
                else:
                    emit_tile(q, hh, b, 0, HH, shifted=sh)

    nc.finalize()
    return nc


def _prep_inputs(inputs, kernel, bias):
    import ml_dtypes

    # fake-quant: integer part exact in bf16, scale folded into eviction
    scale = float(np.max(np.abs(kernel)) / 7.0)
    w_int = np.round(kernel[0] / scale).astype(np.float32)  # [SW, 576, F]

    # weight layout: [128 partitions, SW*9*F]; partition p holds channel
    # c = p % 64 (duplicated across halves for the two PE row groups)
    # free index (b*9 + t)*F + f  ->  w_int[b, c*9 + t, f]
    w4 = w_int.reshape(SW, C, 9, F)                    # [b, c, t, f]
    wt_half = np.transpose(w4, (1, 0, 2, 3)).reshape(C, SW * 9 * F)
    wts_host = np.concatenate([wt_half, wt_half], axis=0)  # [128, SW*9*F]
    wts_host = wts_host.astype(ml_dtypes.bfloat16)

    # input: pad spatially, bf16, pack image pairs into 128 partitions
    xp = np.zeros((N, C, HP, WP), np.float32)
    xp[:, :, 1:-1, 1:-1] = inputs
    xp = xp.astype(ml_dtypes.bfloat16).reshape(N, C, HP * WP)

    bsc_host = np.zeros((128, 2), np.float32)
    bsc_host[:, 0] = np.ascontiguousarray(bias, np.float32)
    bsc_host[:, 1] = scale

    in_maps = []
    for core in range(N_CORES):
        base = core * IMGS_PER_CORE
        xin = np.empty((PAIRS, 128, HP * WP), ml_dtypes.bfloat16)
        for q in range(PAIRS):
            xin[q, 0:64] = xp[base + 2 * q]
            xin[q, 64:128] = xp[base + 2 * q + 1]
        in_maps.append({
            "xin": xin,
            "wts": wts_host,
            "bsc": bsc_host,
        })
    return in_maps


def kernel(inputs, kernel, bias, _trace=False):
    from concourse.bass_utils import run_bass_kernel_spmd

    inputs = np.asarray(inputs)
    kernel = np.asarray(kernel)
    bias = np.asarray(bias)

    if "nc" not in _COMPILED:
        _COMPILED["nc"] = _build_program()
    nc = _COMPILED["nc"]

    in_maps = _prep_inputs(inputs, kernel, bias)
    res = run_bass_kernel_spmd(
        nc, in_maps, list(range(N_CORES)), trace=_trace
    )
    out = np.empty((N, F, H, W), np.float32)
    for core in range(N_CORES):
        base = core * IMGS_PER_CORE
        # y is b-major: [img, f, b, h*WB+wb] -> [img, f, h, b*WB+wb]
        yc = res.results[core]["y"].astype(np.float32)
        yc = yc.reshape(IMGS_PER_CORE, F, SW, H, WB).transpose(0, 1, 3, 2, 4)
        out[base:base + IMGS_PER_CORE] = yc.reshape(IMGS_PER_CORE, F, H, W)
    if _trace:
        return out, res
    return out
